# revision 28
# baseline (speedup 1.0000x reference)
"""Trainium2 Bass kernel for nn_CoAttention_TextDNS.

Math: both additive co-attention blocks have scores of the form
    score[l, m] = f(l) + g(m) + const
followed by softmax over the last axis, so the row-dependent terms cancel
(softmax shift invariance) and the attention weights are identical for every
row l:
    att_dns[b]  = broadcast_rows( softmax(tanh(dns[b]  @ W_d1.T) @ wb) @ dns[b] )
    att_text[b] = broadcast_rows( softmax(tanh(text[b] @ W_t2.T) @ wd) @ text[b] )
with wb = w_att1[H:], wd = w_att2[H:].  W_t1/b_t1/W_d2/b_d2/wa/wc/b_att1/
b_att2 do not affect the output.

Sharding: data-parallel over batch, one batch element per NeuronCore (B=8).
The host pre-transposes every matmul operand so all device DMAs are
contiguous [128, N] block loads; the device computes the two matmuls
(fp32r on the PE), tanh, the wb/wd projections, softmax, and the weighted
row-sums v1/v2; the host broadcasts those rows back to the full
(8, 256, 768) outputs.
"""

import numpy as np

B, L, M, H = 8, 256, 128, 768
HC = H // 128  # 6 contraction chunks of 128


def _build_module(reps=1):
    """Build the per-core module. reps>1 wraps the main pipeline in an
    on-device hardware loop — used only for wall-clock benchmarking (the
    ~70 ms axon dispatch RTT swamps a single ~20 us execution)."""
    import concourse.bass as bass
    import concourse.tile as tile
    from concourse import bacc, mybir
    from concourse.masks import make_identity
    from contextlib import nullcontext

    f32 = mybir.dt.float32
    f32r = mybir.dt.float32r

    nc = bacc.Bacc("TRN2", target_bir_lowering=False, debug=False)

    # Per-core inputs (host-prepared layouts, pre-rounded to the tf32 grid;
    # see kernel()).  PE-consumed tensors are float32r end-to-end so the BIR
    # verifier's fp32r-producer rule is satisfied; the DVE reads them through
    # a bitcast back to plain f32.
    dnst = nc.dram_tensor("dnst", [128, HC * M], f32r, kind="ExternalInput").ap()
    textt = nc.dram_tensor("textt", [128, HC * L], f32r, kind="ExternalInput").ap()
    wd1t = nc.dram_tensor("wd1t", [128, HC * H], f32r, kind="ExternalInput").ap()
    wt2t = nc.dram_tensor("wt2t", [128, HC * H], f32r, kind="ExternalInput").ap()
    wb_in = nc.dram_tensor("wb", [1, H], f32r, kind="ExternalInput").ap()
    wd_in = nc.dram_tensor("wd", [1, H], f32r, kind="ExternalInput").ap()
    v1_out = nc.dram_tensor("v1", [128, HC], f32, kind="ExternalOutput").ap()
    v2_out = nc.dram_tensor("v2", [128, HC], f32, kind="ExternalOutput").ap()

    Tanh = mybir.ActivationFunctionType.Tanh
    Exp = mybir.ActivationFunctionType.Exp

    with tile.TileContext(nc) as tc:
        with (
            tc.tile_pool(name="ins", bufs=1) as ins,
            tc.tile_pool(name="work", bufs=1) as work,
            tc.tile_pool(name="scratch", bufs=2) as scratch,
            tc.tile_pool(name="mm", bufs=2, space="PSUM") as mm,
            tc.tile_pool(name="smallp", bufs=1, space="PSUM") as smallp,
            tc.tile_pool(name="prepp", bufs=2, space="PSUM") as prepp,
            tc.tile_pool(name="urowp", bufs=2, space="PSUM") as urowp,
        ):
            # ---- constants / small inputs -------------------------------
            ident = ins.tile([128, 128], f32, tag="ident")
            make_identity(nc, ident)
            ones_row = ins.tile([1, 128], f32, tag="ones")
            nc.vector.memset(ones_row, 1.0)

            wb_sb = ins.tile([1, H], f32r, tag="wb")
            nc.sync.dma_start(out=wb_sb, in_=wb_in)
            wd_sb = ins.tile([1, H], f32r, tag="wd")
            nc.sync.dma_start(out=wd_sb, in_=wd_in)

            # Replicate wb/wd across all 128 partitions: ones[1,128].T @ w[1,N]
            # on the PE, then park in SBUF.
            wb_rep = work.tile([128, H], f32, tag="wb_rep")
            wd_rep = work.tile([128, H], f32, tag="wd_rep")
            for w_sb, w_rep in ((wb_sb, wb_rep), (wd_sb, wd_rep)):
                for half in range(2):
                    sl = slice(half * 384, (half + 1) * 384)
                    rep_ps = smallp.tile([128, 384], f32, tag="rep")
                    nc.tensor.matmul(
                        rep_ps, ones_row, w_sb[:, sl].bitcast(f32),
                        start=True, stop=True,
                    )
                    nc.scalar.copy(w_rep[:, sl], rep_ps)

            # ---- main pipeline (optionally looped for benchmarking) -----
            loop = tc.For_i(0, reps, 1) if reps > 1 else nullcontext()
            with loop:
                _pipeline_body(nc, tc, ins, work, scratch, mm, prepp, urowp,
                               mybir, dnst, textt, wd1t, wt2t, v1_out, v2_out,
                               ident, ones_row, wb_rep, wd_rep, Tanh, Exp,
                               f32, f32r)

    nc.compile()
    return nc


def _pipeline_body(nc, tc, ins, work, scratch, mm, prepp, urowp, mybir,
                   dnst, textt, wd1t, wt2t, v1_out, v2_out,
                   ident, ones_row, wb_rep, wd_rep, Tanh, Exp, f32, f32r):
    if True:
        if True:
            # ---- bulk input loads (order = DMA priority) ----------------
            dnst_sb = ins.tile([128, HC, M], f32r, tag="dnst")
            nc.sync.dma_start(out=dnst_sb, in_=dnst.rearrange("p (c m) -> p c m", c=HC))
            wd1_sb = ins.tile([128, HC, H], f32r, tag="wd1")
            wd1_r = wd1t.rearrange("p (c o) -> p c o", c=HC)
            for c in range(HC):
                nc.sync.dma_start(out=wd1_sb[:, c, :], in_=wd1_r[:, c, :])
            textt_sb = ins.tile([128, HC, L], f32r, tag="textt")
            nc.sync.dma_start(
                out=textt_sb, in_=textt.rearrange("p (c l) -> p c l", c=HC)
            )
            wt2_sb = ins.tile([128, HC, H], f32r, tag="wt2")
            wt2_r = wt2t.rearrange("p (c o) -> p c o", c=HC)
            for c in range(HC):
                nc.sync.dma_start(out=wt2_sb[:, c, :], in_=wt2_r[:, c, :])

            # ---- helper: one [128, H] tanh(matmul) + projection ---------
            def mm_tanh_proj(lhsT_chunks, w_sb_chunks, w_rep, ucol):
                """tanh(lhsT.T @ W.T) for one 128-row tile, then project the
                H free-dim against w_rep into ucol [128, 1]."""
                act = work.tile([128, H], f32, tag="act")
                for half in range(2):
                    sl = slice(half * 384, (half + 1) * 384)
                    ps = mm.tile([128, 384], f32, tag="mmps")
                    for c in range(HC):
                        nc.tensor.matmul(
                            ps,
                            lhsT_chunks(c),
                            w_sb_chunks(c)[:, sl],
                            start=(c == 0),
                            stop=(c == HC - 1),
                        )
                    nc.scalar.activation(act[:, sl], ps, Tanh)
                prod = scratch.tile([128, H], f32, tag="prod")
                nc.vector.tensor_mul(prod, act, w_rep)
                nc.vector.reduce_sum(out=ucol, in_=prod, axis=mybir.AxisListType.X)

            # d1 path: scores for the 128 dns rows.
            u1col = work.tile([128, 1], f32, tag="u1col")
            mm_tanh_proj(
                lambda c: dnst_sb[:, c, :], lambda c: wd1_sb[:, c, :], wb_rep, u1col
            )

            # t2 path: scores for the 256 text rows (two 128-row tiles).
            u2col = work.tile([128, 2], f32, tag="u2col")
            for lt in range(2):
                mm_tanh_proj(
                    lambda c: textt_sb[:, c, lt * 128 : (lt + 1) * 128],
                    lambda c: wt2_sb[:, c, :],
                    wd_rep,
                    u2col[:, lt : lt + 1],
                )

            # ---- softmax over the score vectors -------------------------
            def softmax_row(urow_sb, n, prefix):
                """softmax of a [1, n] SBUF row; returns SBUF [1, n]."""
                negmax = work.tile([1, 1], f32, tag=f"{prefix}negmax")
                nc.vector.tensor_reduce(
                    out=negmax, in_=urow_sb, op=mybir.AluOpType.max,
                    axis=mybir.AxisListType.X, negate=True,
                )
                erow = work.tile([1, n], f32, tag=f"{prefix}erow")
                esum = work.tile([1, 1], f32, tag=f"{prefix}esum")
                nc.scalar.activation(
                    out=erow, in_=urow_sb, func=Exp, bias=negmax, scale=1.0,
                    accum_out=esum,
                )
                rsum = work.tile([1, 1], f32, tag=f"{prefix}rsum")
                nc.vector.reciprocal(out=rsum, in_=esum)
                prow = work.tile([1, n], f32, tag=f"{prefix}prow")
                nc.vector.tensor_scalar_mul(prow, erow, rsum)
                return prow

            u1sb = work.tile([1, 128], f32, tag="u1sb")
            u1row = urowp.tile([1, 128], f32, tag="urow")
            nc.tensor.transpose(u1row, u1col, ident)
            nc.scalar.copy(u1sb, u1row)
            p1row = softmax_row(u1sb, 128, "p1")

            u2sb = work.tile([1, 256], f32, tag="u2sb")
            for lt in range(2):
                u2row = urowp.tile([1, 128], f32, tag="urow")
                nc.tensor.transpose(u2row, u2col[:, lt : lt + 1], ident)
                nc.scalar.copy(u2sb[:, lt * 128 : (lt + 1) * 128], u2row)
            p2row = softmax_row(u2sb, 256, "p2")

            # ---- weighted row sums v = p @ X ----------------------------
            def weighted_rowsum(prow, n, xt_sb, v_sb):
                """v[h] = sum_r p[r] * X[r, h] given X.T chunks [128, n]."""
                prep = prepp.tile([128, n], f32, tag="prep")
                nc.tensor.matmul(prep, ones_row, prow, start=True, stop=True)
                for c in range(HC):
                    prod = scratch.tile([128, L], f32, tag="vprod")
                    nc.vector.tensor_mul(prod[:, :n], xt_sb[:, c, :].bitcast(f32), prep)
                    nc.vector.reduce_sum(
                        out=v_sb[:, c : c + 1], in_=prod[:, :n],
                        axis=mybir.AxisListType.X,
                    )

            v1_sb = work.tile([128, HC], f32, tag="v1sb")
            weighted_rowsum(p1row, 128, dnst_sb, v1_sb)
            v2_sb = work.tile([128, HC], f32, tag="v2sb")
            weighted_rowsum(p2row, 256, textt_sb, v2_sb)

            nc.sync.dma_start(out=v1_out, in_=v1_sb)
            nc.sync.dma_start(out=v2_out, in_=v2_sb)


_NC_CACHE = {}


def _get_module(reps=1):
    if reps not in _NC_CACHE:
        _NC_CACHE[reps] = _build_module(reps)
    return _NC_CACHE[reps]


def _round_tf32(x):
    """Round fp32 to the tf32 grid (10 mantissa bits, round-to-nearest-even)
    so the PE's fp32r read sees already-representable values."""
    u = np.ascontiguousarray(x, np.float32).view(np.uint32)
    r = (u + np.uint32(0x0FFF) + ((u >> np.uint32(13)) & np.uint32(1))) & np.uint32(
        0xFFFFE000
    )
    return r.view(np.float32)


def _chunked_T(x, inner):
    """[R, H] -> [128, HC*inner] with [p, c*inner + r] = x[r, c*128 + p]."""
    r = x.shape[0]
    assert x.shape == (r, H) and r == inner
    return _round_tf32(
        x.T.reshape(HC, 128, inner).transpose(1, 0, 2).reshape(128, HC * inner)
    )


def _make_in_maps(kernel_inputs):
    text = np.asarray(kernel_inputs["text_features"], np.float32)
    dns = np.asarray(kernel_inputs["dns_features"], np.float32)
    W_d1 = np.asarray(kernel_inputs["W_d1"], np.float32)
    W_t2 = np.asarray(kernel_inputs["W_t2"], np.float32)
    wb = _round_tf32(np.asarray(kernel_inputs["w_att1"], np.float32)[H:].reshape(1, H))
    wd = _round_tf32(np.asarray(kernel_inputs["w_att2"], np.float32)[H:].reshape(1, H))
    wd1t = _chunked_T(W_d1, H)  # [p, c*H + o] = W_d1[o, c*128 + p]
    wt2t = _chunked_T(W_t2, H)

    in_maps = []
    for b in range(B):
        in_maps.append(
            {
                "dnst": _chunked_T(dns[b], M),
                "textt": _chunked_T(text[b], L),
                "wd1t": wd1t,
                "wt2t": wt2t,
                "wb": wb,
                "wd": wd,
            }
        )
    return in_maps


def _run_device(kernel_inputs):
    from concourse.bass_utils import run_bass_kernel_spmd

    in_maps = _make_in_maps(kernel_inputs)
    nc = _get_module()
    return run_bass_kernel_spmd(nc, in_maps, list(range(B)))


def kernel(**inputs):
    res = _run_device(inputs)
    att_text = np.empty((B, L, H), np.float32)
    att_dns = np.empty((B, L, H), np.float32)
    for b in range(B):
        v1 = res.results[b]["v1"].T.reshape(H)  # [128, HC] -> [H]
        v2 = res.results[b]["v2"].T.reshape(H)
        att_dns[b] = v1[None, :]
        att_text[b] = v2[None, :]
    return att_text, att_dns
